# revision 4
# baseline (speedup 1.0000x reference)
"""MultiHeadAttention TRN2 kernel: 8-way (batch x head-half) sharding.

Core c handles batch b=c//2, heads g*8..g*8+8 where g=c%2.

Structure (per core):
- bf16 inputs/SBUF tiles (full matmul rate at N>=256, half the DMA/SBUF).
- Host lays every operand out so each loads with ONE (or 3-4) large DMA
  descriptors: the HW descriptor-generation engine costs ~0.65us per
  descriptor, so descriptor count, not bytes, dominated the old stream.
- Scores for a head PAIR run as two row-tiled matmuls (contraction dk=64
  on array rows 0-63 / 64-127) that execute concurrently in the PE, and
  share one [128,1024] PSUM tile + one exp ACT instruction (the key mask
  folds into the exp's per-partition bias; masked keys were compacted
  away on the host, padding exps to exactly 0).
- V carries a fused ones-column so softmax denominators fall out of the
  P@V matmul (output row 64).
- Fully software-pipelined emission against the FIFO engine queues:
  scores+exp run two chunks ahead of P@V, projection/FC pieces weave into
  the PE gaps of the ACT-bound attention stream, normalization is
  deferred into the next unit. FC rides one query-tile behind attention.
- Output is bf16, one descriptor per 128-row block; host sums the two
  half-head partials per batch in f32.
"""

import numpy as np
from ml_dtypes import bfloat16

import concourse.mybir as mybir
import concourse.tile as tile
from concourse import bacc
from concourse.bass import ts

F32 = mybir.dt.float32
F32R = mybir.dt.float32r
BF16 = mybir.dt.bfloat16
AF = mybir.ActivationFunctionType

BS, L, D = 4, 2048, 1024
NCORES = 8
H = 8                 # heads per core
DK = 64
HD = H * DK           # 512: head dims per core
LK = 1152             # padded compacted-key length (9 chunks of 128)
NEGB = -30000.0       # masked/padded-key bias (exp underflows to exactly 0)
SCALE = 1.0 / 8.0     # 1/sqrt(DK)
NQT = 4               # query tiles of 512
QT = L // NQT


def _build(lk):
    kck = lk // 128           # key chunks
    nkv = (lk + 511) // 512   # 512-wide column blocks of the compacted keys
    nc = bacc.Bacc()
    # layouts are pre-swizzled on the host: [partition, k-chunk, cols]
    xt = nc.declare_dram_parameter("xt", [NQT, 128, 8, QT], BF16,
                                   isOutput=False)
    xkv = nc.declare_dram_parameter("xkv", [128, 8, lk], BF16, isOutput=False)
    wq = nc.declare_dram_parameter("wq", [128, 8, HD], BF16, isOutput=False)
    wk = nc.declare_dram_parameter("wk", [128, 8, HD], BF16, isOutput=False)
    wv = nc.declare_dram_parameter("wv", [128, 8, HD], BF16, isOutput=False)
    wfc = nc.declare_dram_parameter("wfc", [128, 4, D], BF16, isOutput=False)
    bq = nc.declare_dram_parameter("bq", [128, 4], F32, isOutput=False)
    bk = nc.declare_dram_parameter("bk", [128, 4], F32, isOutput=False)
    bvr = nc.declare_dram_parameter("bvr", [1, HD], F32R, isOutput=False)
    bfch = nc.declare_dram_parameter("bfch", [128, 8], F32, isOutput=False)
    mb = nc.declare_dram_parameter("mb", [128, kck], F32, isOutput=False)
    outp = nc.declare_dram_parameter("out", [8, 128, L], BF16, isOutput=True)

    with tile.TileContext(nc) as tc:
        with tc.tile_pool(name="const", bufs=1) as pc, \
             tc.tile_pool(name="w", bufs=1) as p_w, \
             tc.tile_pool(name="qt", bufs=4) as p_qt, \
             tc.tile_pool(name="kt", bufs=4) as p_kt, \
             tc.tile_pool(name="v", bufs=kck) as p_v, \
             tc.tile_pool(name="ctx", bufs=4) as p_ctx, \
             tc.tile_pool(name="pt", bufs=4, side="right") as p_pt, \
             tc.tile_pool(name="smallB", bufs=2, side="right") as p_sm, \
             tc.tile_pool(name="ev", bufs=1, side="right") as p_ev, \
             tc.tile_pool(name="ps", bufs=2, space="PSUM") as PS:
            # ---- constants ----
            ones_f = pc.tile([1, 128], F32)
            nc.vector.memset(ones_f[:], 1.0)
            ones_r = pc.tile([1, 128], F32R)
            nc.vector.tensor_copy(ones_r[:], ones_f[:])
            onesv = pc.tile([128, 8, 1], BF16)
            nc.vector.memset(onesv[:], 1.0)

            # ---- input DMAs, in priority order (one FIFO queue) ----
            wk_sb = p_w.tile([128, 8, HD], BF16, tag="wk", name="wk_sb")
            nc.sync.dma_start(out=wk_sb[:], in_=wk[:])
            xkv_sb = p_w.tile([128, 8, lk], BF16, tag="xkv", name="xkv_sb")

            def dma_xkv(n):
                c0 = n * 512
                w = min(512, lk - c0)
                nc.sync.dma_start(out=xkv_sb[:, :, c0:c0 + w],
                                  in_=xkv[:, :, c0:c0 + w])

            dma_xkv(0)
            wv_sb = p_w.tile([128, 8, HD], BF16, tag="wv", name="wv_sb")
            nc.sync.dma_start(out=wv_sb[:], in_=wv[:])
            bv_sb = pc.tile([1, HD], F32R)
            nc.sync.dma_start(out=bv_sb[:], in_=bvr[:])
            wq_sb = p_w.tile([128, 8, HD], BF16, tag="wq", name="wq_sb")
            nc.sync.dma_start(out=wq_sb[:], in_=wq[:])
            xt_sb = [None] * NQT

            def dma_xt(n):
                xn = p_w.tile([128, 8, QT], BF16, tag="xt", bufs=NQT,
                              name=f"xt{n}")
                nc.sync.dma_start(out=xn[:], in_=xt[n])
                xt_sb[n] = xn

            dma_xt(0)
            mb_sb = pc.tile([128, kck], F32)
            nc.sync.dma_start(out=mb_sb[:], in_=mb[:])
            bk_sb = p_w.tile([128, 4], F32, tag="bk", name="bk_sb")
            nc.sync.dma_start(out=bk_sb[:], in_=bk[:])
            bq_sb = p_w.tile([128, 4], F32, tag="bq", name="bq_sb")
            nc.sync.dma_start(out=bq_sb[:], in_=bq[:])
            for n in range(1, nkv):
                dma_xkv(n)
            dma_xt(1)
            bfc_sb = p_w.tile([128, 8], F32, tag="bfc", name="bfc_sb")
            nc.sync.dma_start(out=bfc_sb[:], in_=bfch[:])
            wfc_sb = p_w.tile([128, 4, D], BF16, tag="wfc", name="wfc_sb")
            nc.sync.dma_start(out=wfc_sb[:], in_=wfc[:])
            dma_xt(2)
            dma_xt(3)

            # ---- persistent SBUF tiles ----
            qt_t = [p_qt.tile([128, L], BF16, tag="qt", name=f"qt{j}")
                    for j in range(4)]
            kt_t = [p_kt.tile([128, lk], BF16, tag="kt", name=f"kt{j}")
                    for j in range(4)]
            v_t = [p_v.tile([128, 8, 65], BF16, tag="v", name=f"v{m}")
                   for m in range(kck)]
            ctx_t = [p_ctx.tile([128, L], BF16, tag="ctx", name=f"ctx{j}")
                     for j in range(4)]
            ev_t = [p_ev.tile([128, L], BF16, tag="ev", bufs=8,
                              name=f"ev{m}")
                    for m in range(8)]

            # ---- projection emitters ----
            def k_piece(t, n):
                # kt_t[t][:, block n] (dims 128t.., keys 512n..)
                c0 = n * 512
                w = min(512, lk - c0)
                ps = PS.tile([128, 512], F32, tag="mm", name="psk")
                for k in range(8):
                    nc.tensor.matmul(ps[:, :w], wk_sb[:, k, ts(t, 128)],
                                     xkv_sb[:, k, c0:c0 + w],
                                     start=(k == 0), stop=(k == 7))
                nc.vector.tensor_scalar_add(
                    kt_t[t][:, c0:c0 + w], ps[:, :w], bk_sb[:, t:t + 1])

            def v_chunk(m):
                ps = PS.tile([128, 512], F32, tag="mm", name="psv")
                for k in range(8):
                    nc.tensor.matmul(ps[:], xkv_sb[:, k, ts(m, 128)],
                                     wv_sb[:, k, :], start=(k == 0),
                                     stop=False)
                nc.tensor.matmul(ps[:], ones_r[:, :128], bv_sb[:],
                                 start=False, stop=True)
                nc.vector.tensor_copy(
                    v_t[m][:, :, 0:64],
                    ps[:].rearrange("p (h d) -> p h d", h=8))
                nc.vector.tensor_copy(v_t[m][:, :, 64:65], onesv[:])

            def q_piece(j, n):
                # qt_t[j][:, n*QT:(n+1)*QT] (pair j dims, query tile n)
                ps = PS.tile([128, 512], F32, tag="mm", name="psq")
                for k in range(8):
                    nc.tensor.matmul(ps[:], wq_sb[:, k, ts(j, 128)],
                                     xt_sb[n][:, k, :], start=(k == 0),
                                     stop=(k == 7))
                nc.vector.tensor_scalar_add(qt_t[j][:, ts(n, QT)],
                                            ps[:], bq_sb[:, j:j + 1])

            def fc_piece(n, m):
                # out rows 128m..128m+127, query tile n
                ps = PS.tile([128, 512], F32, tag="mm", name="psf")
                for k in range(4):
                    nc.tensor.matmul(ps[:], wfc_sb[:, k, ts(m, 128)],
                                     ctx_t[k][:, ts(n, QT)],
                                     start=(k == 0), stop=(k == 3))
                nc.vector.tensor_scalar_add(ev_t[m][:, ts(n, QT)], ps[:],
                                            bfc_sb[:, m:m + 1])
                if n == NQT - 1:
                    nc.sync.dma_start(out=outp[m][:], in_=ev_t[m][:])

            # ---- attention unit: head pair j, query tile n ----
            # Software-pipelined at chunk granularity against the PE/ACT
            # FIFO queues: scores+exp run two chunks ahead of PV, fill
            # closures (projection/FC pieces) are woven between, and the
            # softmax normalization is deferred into the next unit so the
            # PE never sits behind an unfinished exp at its queue head.
            def attn_unit(j, n, fills_at, norm_prev):
                q0 = n * QT
                cA = PS.tile([65, 512], F32, tag="ctxp", name="cA")
                cB = PS.tile([65, 512], F32, tag="ctxp", name="cB")
                pts = {}

                def scexp(kc):
                    sps = PS.tile([128, 1024], F32, tag="s", name="s")
                    nc.tensor.matmul(sps[:, 0:512],
                                     kt_t[j][0:64, ts(kc, 128)],
                                     qt_t[j][0:64, q0:q0 + 512],
                                     start=True, stop=True)
                    nc.tensor.matmul(sps[:, 512:1024],
                                     kt_t[j][64:128, ts(kc, 128)],
                                     qt_t[j][64:128, q0:q0 + 512],
                                     start=True, stop=True)
                    pt = p_pt.tile([128, 1024], BF16, tag="pt", name="pt")
                    nc.scalar.activation(pt[:], sps[:], AF.Exp,
                                         bias=mb_sb[:, kc:kc + 1], scale=SCALE)
                    pts[kc] = pt

                scexp(0)
                scexp(1)
                if norm_prev is not None:
                    norm_prev()
                for kc in range(kck):
                    if kc + 2 < kck:
                        scexp(kc + 2)
                    for f in fills_at.get(kc, ()):
                        f()
                    pt = pts.pop(kc)
                    st, sp = (kc == 0), (kc == kck - 1)
                    nc.tensor.matmul(cA[:], v_t[kc][:, 2 * j, :],
                                     pt[:, 0:512], start=st, stop=sp)
                    nc.tensor.matmul(cB[:], v_t[kc][:, 2 * j + 1, :],
                                     pt[:, 512:1024], start=st, stop=sp)

                def norm():
                    for half, cp in ((0, cA), (1, cB)):
                        den = p_sm.tile([1, 512], F32R, tag="den", name="den")
                        nc.vector.tensor_copy(den[:], cp[64:65, :])
                        rbps = PS.tile([64, 512], F32, tag="mm", name="rbps")
                        nc.tensor.matmul(rbps[:], ones_r[:, 0:64], den[:],
                                         start=True, stop=True)
                        rbs = p_sm.tile([64, 512], F32, tag="rbs", name="rbs")
                        nc.vector.reciprocal(rbs[:], rbps[:])
                        oh = half * 64
                        nc.vector.tensor_mul(
                            ctx_t[j][oh:oh + 64, q0:q0 + 512],
                            cp[0:64, :], rbs[:])

                return norm

            # ---- pipelined emission ----
            k_piece(0, 0)
            v_chunk(0)
            q_piece(0, 0)

            def spread(fills):
                out = {}
                for i, f in enumerate(fills):
                    out.setdefault(3 + (i * 6) // max(len(fills), 1), []
                                   ).append(f)
                return out

            unit_fills = {
                0: {kc: [lambda m=kc + 1: v_chunk(m)] for kc in range(kck - 1)},
                3: spread([lambda: q_piece(0, 1)]),
            }
            # unit 0 also carries the rest of kt0, kt1 and the next q proj
            for kc, f in ((1, lambda: k_piece(0, 1)),
                          (4, lambda: k_piece(0, 2)),
                          (5, lambda: k_piece(1, 0)),
                          (6, lambda: k_piece(1, 1)),
                          (7, lambda: k_piece(1, 2)),
                          (8, lambda: q_piece(1, 0))):
                unit_fills[0].setdefault(kc, []).append(f)
            unit_fills[1] = spread([lambda: k_piece(2, 0),
                                    lambda: k_piece(2, 1),
                                    lambda: k_piece(2, 2),
                                    lambda: q_piece(2, 0)])
            unit_fills[2] = spread([lambda: k_piece(3, 0),
                                    lambda: k_piece(3, 1),
                                    lambda: k_piece(3, 2),
                                    lambda: q_piece(3, 0)])
            for u in range(4, 16):
                r, jj = (u // 4), (u % 4)
                fl = []
                qn = u + 1
                if qn < 16:
                    fl.append(lambda j=qn % 4, n=qn // 4: q_piece(j, n))
                fl.append(lambda n=r - 1, m=2 * jj: fc_piece(n, m))
                fl.append(lambda n=r - 1, m=2 * jj + 1: fc_piece(n, m))
                unit_fills[u] = spread(fl)

            norm_prev = None
            for u in range(16):
                r, jj = u // 4, u % 4
                norm_prev = attn_unit(jj, r, unit_fills[u], norm_prev)
            norm_prev()
            for m in range(8):
                fc_piece(3, m)

    nc.finalize()
    return nc


class _Runner:
    """Compile-once wrapper around the run_bass_via_pjrt shard_map path."""

    def __init__(self, nc):
        import jax
        from jax.sharding import Mesh, PartitionSpec

        from concourse import bass2jax, mybir as mb

        try:
            from jax.experimental.shard_map import shard_map
        except ImportError:
            from jax.shard_map import shard_map

        bass2jax.install_neuronx_cc_hook()
        self._nc = nc
        partition_name = (nc.partition_id_tensor.name
                          if nc.partition_id_tensor else None)
        in_names, out_names, out_avals = [], [], []
        self._zero_shapes = []
        in_avals = []
        for alloc in nc.m.functions[0].allocations:
            if not isinstance(alloc, mb.MemoryLocationSet):
                continue
            name = alloc.memorylocations[0].name
            if alloc.kind == "ExternalInput":
                if name != partition_name:
                    in_names.append(name)
                    in_avals.append((tuple(alloc.tensor_shape),
                                     mb.dt.np(alloc.dtype)))
            elif alloc.kind == "ExternalOutput":
                out_names.append(name)
                shape = tuple(alloc.tensor_shape)
                dtype = mb.dt.np(alloc.dtype)
                out_avals.append(jax.core.ShapedArray(shape, dtype))
                self._zero_shapes.append((shape, dtype))
        self._n_params = len(in_names)
        n_outs = len(out_avals)
        self._in_names = list(in_names)
        self._out_names = list(out_names)
        self._out_avals = out_avals
        all_in = in_names + out_names
        if partition_name is not None:
            all_in.append(partition_name)

        def _body(*args):
            operands = list(args)
            if partition_name is not None:
                operands.append(bass2jax.partition_id_tensor())
            return tuple(bass2jax._bass_exec_p.bind(
                *operands,
                out_avals=tuple(out_avals),
                in_names=tuple(all_in),
                out_names=tuple(out_names),
                lowering_input_output_aliases=(),
                sim_require_finite=True,
                sim_require_nnan=True,
                nc=nc,
            ))

        devices = jax.devices()[:NCORES]
        mesh = Mesh(np.asarray(devices), ("core",))
        self.mesh = mesh
        nin = self._n_params + n_outs

        def compile_fn():
            jitted = jax.jit(
                shard_map(_body, mesh=mesh,
                          in_specs=(PartitionSpec("core"),) * nin,
                          out_specs=(PartitionSpec("core"),) * n_outs,
                          check_rep=False),
                donate_argnums=tuple(range(self._n_params, nin)),
                keep_unused=True,
            )
            args = [np.zeros((NCORES * s[0], *s[1:]), d)
                    for s, d in in_avals]
            args += [np.zeros((NCORES * s[0], *s[1:]), d)
                     for s, d in self._zero_shapes]
            return jitted.lower(*args).compile()

        try:
            self._sharded = bass2jax.fast_dispatch_compile(compile_fn)
        except Exception:
            self._sharded = compile_fn()

    def run(self, in_maps):
        import jax
        concat_in = [
            np.concatenate([np.asarray(in_maps[c][name])
                            for c in range(NCORES)], axis=0)
            for name in self._in_names
        ]
        concat_zeros = [np.zeros((NCORES * s[0], *s[1:]), d)
                        for s, d in self._zero_shapes]
        out_arrs = self._sharded(*concat_in, *concat_zeros)
        jax.block_until_ready(out_arrs)
        return [
            {name: np.asarray(out_arrs[i]).reshape(
                NCORES, *self._out_avals[i].shape)[c]
             for i, name in enumerate(self._out_names)}
            for c in range(NCORES)
        ]


_RUNNERS = {}


def _get_runner(lk):
    if lk not in _RUNNERS:
        _RUNNERS[lk] = _Runner(_build(lk))
    return _RUNNERS[lk]


def _swiz(a, nchunk):
    """[nchunk*128, C] -> [128, nchunk, C] host swizzle (partition-major)."""
    c = a.shape[1]
    return np.ascontiguousarray(
        a.reshape(nchunk, 128, c).transpose(1, 0, 2))


def _prep_in_maps(x, mask, Wq, bq, Wk, bk, Wv, bv, Wfc, bfc):
    """Shard + lay out the full inputs for the 8 cores.

    Returns (in_maps, lk) or (None, None) if the mask leaves more than LK
    keys unmasked in some batch (host fallback).
    """
    keep = [np.nonzero(mask[b] == 0)[0] for b in range(BS)]
    if max(len(kp) for kp in keep) > LK or min(len(kp) for kp in keep) == 0:
        return None, None
    lk = LK

    in_maps = []
    for c in range(NCORES):
        b, g = c // 2, c % 2
        sl = slice(g * HD, (g + 1) * HD)
        kp = keep[b]
        xkv_b = np.zeros((lk, D), np.float32)
        xkv_b[:len(kp)] = x[b][kp]
        biask = np.where(np.arange(lk) < len(kp), 0.0, NEGB).astype(np.float32)
        xT = np.ascontiguousarray(x[b].T)           # [D, L]
        # xt: [NQT, 128, 8, QT]
        xt_l = xT.reshape(8, 128, NQT, QT).transpose(2, 1, 0, 3)
        in_maps.append({
            "xt": np.ascontiguousarray(xt_l).astype(bfloat16),
            "xkv": _swiz(xkv_b.T, 8).astype(bfloat16),
            "wq": _swiz(np.ascontiguousarray(Wq[:, sl]), 8).astype(bfloat16),
            "wk": _swiz(np.ascontiguousarray(Wk[:, sl]), 8).astype(bfloat16),
            "wv": _swiz(np.ascontiguousarray(Wv[:, sl]), 8).astype(bfloat16),
            "wfc": _swiz(np.ascontiguousarray(Wfc[sl, :]), 4).astype(bfloat16),
            "bq": np.ascontiguousarray(bq[sl].reshape(4, 128).T),
            "bk": np.ascontiguousarray(bk[sl].reshape(4, 128).T),
            "bvr": np.ascontiguousarray(bv[sl]).reshape(1, HD),
            "bfch": np.ascontiguousarray((bfc * 0.5).reshape(8, 128).T),
            "mb": np.ascontiguousarray(biask.reshape(lk // 128, 128).T),
        })
    return in_maps, lk


def _host_reference(x, mask, Wq, bq, Wk, bk, Wv, bv, Wfc, bfc):
    """Numpy fallback, bit-compatible with the reference semantics."""
    out = np.empty((BS, L, D), np.float32)
    for b in range(BS):
        q = (x[b] @ Wq + bq).reshape(L, 16, DK).transpose(1, 0, 2)
        k = (x[b] @ Wk + bk).reshape(L, 16, DK).transpose(1, 0, 2)
        v = (x[b] @ Wv + bv).reshape(L, 16, DK).transpose(1, 0, 2)
        s = np.einsum("hqd,hkd->hqk", q, k) * SCALE
        m = mask[b].astype(np.float32)[None, None, :]
        s = s * (1.0 - m) + m * (-1e30)
        s = s - s.max(axis=-1, keepdims=True)
        p = np.exp(s)
        p /= p.sum(axis=-1, keepdims=True)
        o = np.einsum("hqk,hkd->hqd", p, v).transpose(1, 0, 2).reshape(L, D)
        out[b] = o @ Wfc + bfc
    return out


def kernel(x, mask, Wq, bq, Wk, bk, Wv, bv, Wfc, bfc, **_unused):
    x = np.asarray(x, np.float32)
    mask = np.asarray(mask)
    Wq, bq = np.asarray(Wq, np.float32), np.asarray(bq, np.float32)
    Wk, bk = np.asarray(Wk, np.float32), np.asarray(bk, np.float32)
    Wv, bv = np.asarray(Wv, np.float32), np.asarray(bv, np.float32)
    Wfc, bfc = np.asarray(Wfc, np.float32), np.asarray(bfc, np.float32)

    in_maps, lk = _prep_in_maps(x, mask, Wq, bq, Wk, bk, Wv, bv, Wfc, bfc)
    if in_maps is None:
        return _host_reference(x, mask, Wq, bq, Wk, bk, Wv, bv, Wfc, bfc)
    results = _get_runner(lk).run(in_maps)

    out = np.empty((BS, L, D), np.float32)
    for b in range(BS):
        p0 = results[2 * b]["out"].reshape(D, L).astype(np.float32)
        p1 = results[2 * b + 1]["out"].reshape(D, L).astype(np.float32)
        out[b] = (p0 + p1).T
    return out


# revision 8
# speedup vs baseline: 1.7440x; 1.7440x over previous
"""MultiHeadAttention TRN2 kernel: 8-way (batch x head-half) sharding.

Core c handles batch b=c//2, heads g*8..g*8+8 where g=c%2.

Structure (per core):
- bf16 inputs/SBUF tiles (full matmul rate at N>=256, half the DMA/SBUF).
- Host lays every operand out so each loads with ONE (or 3-4) large DMA
  descriptors: the HW descriptor-generation engine costs ~0.65us per
  descriptor, so descriptor count, not bytes, dominated the old stream.
- Scores for a head PAIR run as two row-tiled matmuls (contraction dk=64
  on array rows 0-63 / 64-127) that execute concurrently in the PE, and
  share one [128,1024] PSUM tile + one exp ACT instruction (the key mask
  folds into the exp's per-partition bias; masked keys were compacted
  away on the host, padding exps to exactly 0).
- V carries a fused ones-column so softmax denominators fall out of the
  P@V matmul (output row 64).
- Fully software-pipelined emission against the FIFO engine queues:
  scores+exp run two chunks ahead of P@V, projection/FC pieces weave into
  the PE gaps of the ACT-bound attention stream, normalization is
  deferred into the next unit. FC rides one query-tile behind attention.
- Output is bf16, one descriptor per 128-row block; host sums the two
  half-head partials per batch in f32.
"""

import numpy as np
from ml_dtypes import bfloat16

import concourse.mybir as mybir
import concourse.tile as tile
from concourse import bacc
from concourse.bass import ts

F32 = mybir.dt.float32
F32R = mybir.dt.float32r
BF16 = mybir.dt.bfloat16
AF = mybir.ActivationFunctionType

BS, L, D = 4, 2048, 1024
NCORES = 8
H = 8                 # heads per core
DK = 64
HD = H * DK           # 512: head dims per core
LK = 1152             # padded compacted-key length (9 chunks of 128)
NEGB = -30000.0       # masked/padded-key bias (exp underflows to exactly 0)
SCALE = 1.0 / 8.0     # 1/sqrt(DK)
NQT = 4               # query tiles of 512
QT = L // NQT


def _build(lk):
    kck = lk // 128           # key chunks
    nkv = (lk + 511) // 512   # 512-wide column blocks of the compacted keys
    nc = bacc.Bacc()
    # Two input blobs (buffer-binding costs ~42us per input per exec):
    # bf16 blob cols: [xt: NQT*8*QT][xkv blocks: 8*512,8*512,8*(lk-1024)]
    #                 [wq: 8*HD][wk: 8*HD][wv: 8*HD][wfc: 4*D]
    # f32 blob cols:  [bq:4][bk:4][bfc:8][mb:kck][bv: HD (partition 0 only)]
    xt_cols = NQT * 8 * QT
    xkv_off = [0, 8 * 512, 8 * 1024]
    xkv_cols = 8 * lk
    cb = xt_cols + xkv_cols + 3 * (8 * HD) + 4 * D
    cf = 4 + 4 + 8 + kck + HD
    bin_ = nc.declare_dram_parameter("bin", [128, cb], BF16, isOutput=False)
    fin = nc.declare_dram_parameter("fin", [128, cf], F32, isOutput=False)
    OXT, OXKV = 0, xt_cols
    OWQ = OXKV + xkv_cols
    OWK, OWV = OWQ + 8 * HD, OWQ + 2 * (8 * HD)
    OWFC = OWQ + 3 * (8 * HD)
    FBQ, FBK, FBFC, FMB, FBV = 0, 4, 8, 16, 16 + kck
    outp = nc.declare_dram_parameter("out", [8, 128, L], BF16, isOutput=True)

    with tile.TileContext(nc) as tc:
        with tc.tile_pool(name="const", bufs=1) as pc, \
             tc.tile_pool(name="w", bufs=1) as p_w, \
             tc.tile_pool(name="qt", bufs=4) as p_qt, \
             tc.tile_pool(name="kt", bufs=4) as p_kt, \
             tc.tile_pool(name="v", bufs=kck) as p_v, \
             tc.tile_pool(name="ctx", bufs=4) as p_ctx, \
             tc.tile_pool(name="pt", bufs=4, side="right") as p_pt, \
             tc.tile_pool(name="smallB", bufs=2, side="right") as p_sm, \
             tc.tile_pool(name="ev", bufs=1, side="right") as p_ev, \
             tc.tile_pool(name="ps", bufs=2, space="PSUM") as PS:
            # ---- constants ----
            ones_f = pc.tile([1, 128], F32)
            nc.vector.memset(ones_f[:], 1.0)
            ones_r = pc.tile([1, 128], F32R)
            nc.vector.tensor_copy(ones_r[:], ones_f[:])
            onesv = pc.tile([128, 8, 1], BF16)
            nc.vector.memset(onesv[:], 1.0)

            # ---- input DMAs, in priority order (one FIFO queue) ----
            bw = [512, 512, lk - 1024]
            wk_sb = p_w.tile([128, 8, HD], BF16, tag="wk", name="wk_sb")
            nc.sync.dma_start(out=wk_sb[:], in_=bin_[:, OWK:OWK + 8 * HD])
            xkv_b = []
            for n in range(nkv):
                xb = p_w.tile([128, 8, bw[n]], BF16, tag=f"xkv{n}",
                              name=f"xkv_b{n}")
                xkv_b.append(xb)

            def dma_xkv(n):
                o = OXKV + xkv_off[n]
                nc.sync.dma_start(out=xkv_b[n][:],
                                  in_=bin_[:, o:o + 8 * bw[n]])

            dma_xkv(0)
            wv_sb = p_w.tile([128, 8, HD], BF16, tag="wv", name="wv_sb")
            nc.sync.dma_start(out=wv_sb[:], in_=bin_[:, OWV:OWV + 8 * HD])
            bv_f = pc.tile([1, HD], F32)
            nc.sync.dma_start(out=bv_f[:], in_=fin[0:1, FBV:FBV + HD])
            bv_sb = pc.tile([1, HD], F32R)
            nc.vector.tensor_copy(bv_sb[:], bv_f[:])
            wq_sb = p_w.tile([128, 8, HD], BF16, tag="wq", name="wq_sb")
            nc.sync.dma_start(out=wq_sb[:], in_=bin_[:, OWQ:OWQ + 8 * HD])
            xt_sb = [None] * NQT

            def dma_xt(n):
                xn = p_w.tile([128, 8, QT], BF16, tag="xt", bufs=NQT,
                              name=f"xt{n}")
                o = OXT + n * 8 * QT
                nc.sync.dma_start(out=xn[:], in_=bin_[:, o:o + 8 * QT])
                xt_sb[n] = xn

            dma_xt(0)
            mb_sb = pc.tile([128, kck], F32)
            nc.sync.dma_start(out=mb_sb[:], in_=fin[:, FMB:FMB + kck])
            bk_sb = p_w.tile([128, 4], F32, tag="bk", name="bk_sb")
            nc.sync.dma_start(out=bk_sb[:], in_=fin[:, FBK:FBK + 4])
            bq_sb = p_w.tile([128, 4], F32, tag="bq", name="bq_sb")
            nc.sync.dma_start(out=bq_sb[:], in_=fin[:, FBQ:FBQ + 4])
            for n in range(1, nkv):
                dma_xkv(n)
            dma_xt(1)
            bfc_sb = p_w.tile([128, 8], F32, tag="bfc", name="bfc_sb")
            nc.sync.dma_start(out=bfc_sb[:], in_=fin[:, FBFC:FBFC + 8])
            wfc_sb = p_w.tile([128, 4, D], BF16, tag="wfc", name="wfc_sb")
            nc.sync.dma_start(out=wfc_sb[:], in_=bin_[:, OWFC:OWFC + 4 * D])
            dma_xt(2)
            dma_xt(3)

            # ---- persistent SBUF tiles ----
            qt_t = [p_qt.tile([128, L], BF16, tag="qt", name=f"qt{j}")
                    for j in range(4)]
            kt_t = [p_kt.tile([128, lk], BF16, tag="kt", name=f"kt{j}")
                    for j in range(4)]
            v_t = [p_v.tile([128, 8, 65], BF16, tag="v", name=f"v{m}")
                   for m in range(kck)]
            ctx_t = [p_ctx.tile([128, L], BF16, tag="ctx", name=f"ctx{j}")
                     for j in range(4)]
            ev_t = [p_ev.tile([128, L], BF16, tag="ev", bufs=8,
                              name=f"ev{m}")
                    for m in range(8)]

            # ---- projection emitters ----
            def k_piece(t, n):
                # kt_t[t][:, block n] (dims 128t.., keys 512n..)
                c0 = n * 512
                w = min(512, lk - c0)
                ps = PS.tile([128, 512], F32, tag="mm", name="psk")
                for k in range(8):
                    nc.tensor.matmul(ps[:, :w], wk_sb[:, k, ts(t, 128)],
                                     xkv_b[n][:, k, :w],
                                     start=(k == 0), stop=(k == 7))
                nc.vector.tensor_scalar_add(
                    kt_t[t][:, c0:c0 + w], ps[:, :w], bk_sb[:, t:t + 1])

            def v_chunk(m):
                ps = PS.tile([128, 512], F32, tag="mm", name="psv")
                n_, mi = m // 4, m % 4
                for k in range(8):
                    nc.tensor.matmul(ps[:], xkv_b[n_][:, k, ts(mi, 128)],
                                     wv_sb[:, k, :], start=(k == 0),
                                     stop=False)
                nc.tensor.matmul(ps[:], ones_r[:, :128], bv_sb[:],
                                 start=False, stop=True)
                nc.vector.tensor_copy(
                    v_t[m][:, :, 0:64],
                    ps[:].rearrange("p (h d) -> p h d", h=8))
                nc.vector.tensor_copy(v_t[m][:, :, 64:65], onesv[:])

            def q_piece(j, n):
                # qt_t[j][:, n*QT:(n+1)*QT] (pair j dims, query tile n)
                ps = PS.tile([128, 512], F32, tag="mm", name="psq")
                for k in range(8):
                    nc.tensor.matmul(ps[:], wq_sb[:, k, ts(j, 128)],
                                     xt_sb[n][:, k, :], start=(k == 0),
                                     stop=(k == 7))
                nc.vector.tensor_scalar_add(qt_t[j][:, ts(n, QT)],
                                            ps[:], bq_sb[:, j:j + 1])

            def fc_piece(n, m):
                # out rows 128m..128m+127, query tile n
                ps = PS.tile([128, 512], F32, tag="mm", name="psf")
                for k in range(4):
                    nc.tensor.matmul(ps[:], wfc_sb[:, k, ts(m, 128)],
                                     ctx_t[k][:, ts(n, QT)],
                                     start=(k == 0), stop=(k == 3))
                nc.vector.tensor_scalar_add(ev_t[m][:, ts(n, QT)], ps[:],
                                            bfc_sb[:, m:m + 1])
                if n == NQT - 1:
                    nc.sync.dma_start(out=outp[m][:], in_=ev_t[m][:])

            # ---- attention unit: head pair j, query tile n ----
            # Software-pipelined at chunk granularity against the PE/ACT
            # FIFO queues: scores+exp run two chunks ahead of PV, fill
            # closures (projection/FC pieces) are woven between, and the
            # softmax normalization is deferred into the next unit so the
            # PE never sits behind an unfinished exp at its queue head.
            def attn_unit(j, n, fills_at, norm_prev):
                q0 = n * QT
                cA = PS.tile([65, 512], F32, tag="ctxp", name="cA")
                cB = PS.tile([65, 512], F32, tag="ctxp", name="cB")
                pts = {}

                def scexp(kc):
                    sps = PS.tile([128, 1024], F32, tag="s", name="s")
                    nc.tensor.matmul(sps[:, 0:512],
                                     kt_t[j][0:64, ts(kc, 128)],
                                     qt_t[j][0:64, q0:q0 + 512],
                                     start=True, stop=True)
                    nc.tensor.matmul(sps[:, 512:1024],
                                     kt_t[j][64:128, ts(kc, 128)],
                                     qt_t[j][64:128, q0:q0 + 512],
                                     start=True, stop=True)
                    pt = p_pt.tile([128, 1024], BF16, tag="pt", name="pt")
                    nc.scalar.activation(pt[:], sps[:], AF.Exp,
                                         bias=mb_sb[:, kc:kc + 1], scale=SCALE)
                    pts[kc] = pt

                scexp(0)
                scexp(1)
                if norm_prev is not None:
                    norm_prev()
                for kc in range(kck):
                    if kc + 2 < kck:
                        scexp(kc + 2)
                    for f in fills_at.get(kc, ()):
                        f()
                    pt = pts.pop(kc)
                    st, sp = (kc == 0), (kc == kck - 1)
                    nc.tensor.matmul(cA[:], v_t[kc][:, 2 * j, :],
                                     pt[:, 0:512], start=st, stop=sp)
                    nc.tensor.matmul(cB[:], v_t[kc][:, 2 * j + 1, :],
                                     pt[:, 512:1024], start=st, stop=sp)

                def norm():
                    for half, cp in ((0, cA), (1, cB)):
                        den = p_sm.tile([1, 512], F32R, tag="den", name="den")
                        nc.vector.tensor_copy(den[:], cp[64:65, :])
                        rbps = PS.tile([64, 512], F32, tag="mm", name="rbps")
                        nc.tensor.matmul(rbps[:], ones_r[:, 0:64], den[:],
                                         start=True, stop=True)
                        rbs = p_sm.tile([64, 512], F32, tag="rbs", name="rbs")
                        nc.vector.reciprocal(rbs[:], rbps[:])
                        oh = half * 64
                        nc.vector.tensor_mul(
                            ctx_t[j][oh:oh + 64, q0:q0 + 512],
                            cp[0:64, :], rbs[:])

                return norm

            # ---- pipelined emission ----
            k_piece(0, 0)
            v_chunk(0)
            q_piece(0, 0)

            def spread(fills):
                out = {}
                for i, f in enumerate(fills):
                    out.setdefault(3 + (i * 6) // max(len(fills), 1), []
                                   ).append(f)
                return out

            unit_fills = {
                0: {kc: [lambda m=kc + 1: v_chunk(m)] for kc in range(kck - 1)},
                3: spread([lambda: q_piece(0, 1)]),
            }
            # unit 0 also carries the rest of kt0, kt1 and the next q proj
            for kc, f in ((1, lambda: k_piece(0, 1)),
                          (4, lambda: k_piece(0, 2)),
                          (5, lambda: k_piece(1, 0)),
                          (6, lambda: k_piece(1, 1)),
                          (7, lambda: k_piece(1, 2)),
                          (8, lambda: q_piece(1, 0))):
                unit_fills[0].setdefault(kc, []).append(f)
            unit_fills[1] = spread([lambda: k_piece(2, 0),
                                    lambda: k_piece(2, 1),
                                    lambda: k_piece(2, 2),
                                    lambda: q_piece(2, 0)])
            unit_fills[2] = spread([lambda: k_piece(3, 0),
                                    lambda: k_piece(3, 1),
                                    lambda: k_piece(3, 2),
                                    lambda: q_piece(3, 0)])
            for u in range(4, 16):
                r, jj = (u // 4), (u % 4)
                fl = []
                qn = u + 1
                if qn < 16:
                    fl.append(lambda j=qn % 4, n=qn // 4: q_piece(j, n))
                fl.append(lambda n=r - 1, m=2 * jj: fc_piece(n, m))
                fl.append(lambda n=r - 1, m=2 * jj + 1: fc_piece(n, m))
                unit_fills[u] = spread(fl)

            norm_prev = None
            for u in range(16):
                r, jj = u // 4, u % 4
                norm_prev = attn_unit(jj, r, unit_fills[u], norm_prev)
            norm_prev()
            for m in range(8):
                fc_piece(3, m)

    nc.finalize()
    return nc


class _Runner:
    """Compile-once wrapper around the run_bass_via_pjrt shard_map path."""

    def __init__(self, nc):
        import jax
        from jax.sharding import Mesh, PartitionSpec

        from concourse import bass2jax, mybir as mb

        try:
            from jax.experimental.shard_map import shard_map
        except ImportError:
            from jax.shard_map import shard_map

        bass2jax.install_neuronx_cc_hook()
        self._nc = nc
        partition_name = (nc.partition_id_tensor.name
                          if nc.partition_id_tensor else None)
        in_names, out_names, out_avals = [], [], []
        self._zero_shapes = []
        in_avals = []
        for alloc in nc.m.functions[0].allocations:
            if not isinstance(alloc, mb.MemoryLocationSet):
                continue
            name = alloc.memorylocations[0].name
            if alloc.kind == "ExternalInput":
                if name != partition_name:
                    in_names.append(name)
                    in_avals.append((tuple(alloc.tensor_shape),
                                     mb.dt.np(alloc.dtype)))
            elif alloc.kind == "ExternalOutput":
                out_names.append(name)
                shape = tuple(alloc.tensor_shape)
                dtype = mb.dt.np(alloc.dtype)
                out_avals.append(jax.core.ShapedArray(shape, dtype))
                self._zero_shapes.append((shape, dtype))
        self._n_params = len(in_names)
        n_outs = len(out_avals)
        self._in_names = list(in_names)
        self._out_names = list(out_names)
        self._out_avals = out_avals
        all_in = in_names + out_names
        if partition_name is not None:
            all_in.append(partition_name)

        def _body(*args):
            operands = list(args)
            if partition_name is not None:
                operands.append(bass2jax.partition_id_tensor())
            return tuple(bass2jax._bass_exec_p.bind(
                *operands,
                out_avals=tuple(out_avals),
                in_names=tuple(all_in),
                out_names=tuple(out_names),
                lowering_input_output_aliases=(),
                sim_require_finite=True,
                sim_require_nnan=True,
                nc=nc,
            ))

        devices = jax.devices()[:NCORES]
        mesh = Mesh(np.asarray(devices), ("core",))
        self.mesh = mesh
        nin = self._n_params + n_outs

        def compile_fn():
            jitted = jax.jit(
                shard_map(_body, mesh=mesh,
                          in_specs=(PartitionSpec("core"),) * nin,
                          out_specs=(PartitionSpec("core"),) * n_outs,
                          check_rep=False),
                donate_argnums=tuple(range(self._n_params, nin)),
                keep_unused=True,
            )
            args = [np.zeros((NCORES * s[0], *s[1:]), d)
                    for s, d in in_avals]
            args += [np.zeros((NCORES * s[0], *s[1:]), d)
                     for s, d in self._zero_shapes]
            return jitted.lower(*args).compile()

        try:
            self._sharded = bass2jax.fast_dispatch_compile(compile_fn)
        except Exception:
            self._sharded = compile_fn()

    def run(self, in_maps):
        import jax
        concat_in = [
            np.concatenate([np.asarray(in_maps[c][name])
                            for c in range(NCORES)], axis=0)
            for name in self._in_names
        ]
        concat_zeros = [np.zeros((NCORES * s[0], *s[1:]), d)
                        for s, d in self._zero_shapes]
        out_arrs = self._sharded(*concat_in, *concat_zeros)
        jax.block_until_ready(out_arrs)
        return [
            {name: np.asarray(out_arrs[i]).reshape(
                NCORES, *self._out_avals[i].shape)[c]
             for i, name in enumerate(self._out_names)}
            for c in range(NCORES)
        ]


_RUNNERS = {}


def _get_runner(lk):
    if lk not in _RUNNERS:
        _RUNNERS[lk] = _Runner(_build(lk))
    return _RUNNERS[lk]


def _swiz(a, nchunk):
    """[nchunk*128, C] -> [128, nchunk, C] host swizzle (partition-major)."""
    c = a.shape[1]
    return np.ascontiguousarray(
        a.reshape(nchunk, 128, c).transpose(1, 0, 2))


def _prep_in_maps(x, mask, Wq, bq, Wk, bk, Wv, bv, Wfc, bfc):
    """Shard + lay out the full inputs for the 8 cores.

    Returns (in_maps, lk) or (None, None) if the mask leaves more than LK
    keys unmasked in some batch (host fallback).
    """
    keep = [np.nonzero(mask[b] == 0)[0] for b in range(BS)]
    if max(len(kp) for kp in keep) > LK or min(len(kp) for kp in keep) == 0:
        return None, None
    lk = LK

    in_maps = []
    kck = lk // 128
    for c in range(NCORES):
        b, g = c // 2, c % 2
        sl = slice(g * HD, (g + 1) * HD)
        kp = keep[b]
        xkv_b = np.zeros((lk, D), np.float32)
        xkv_b[:len(kp)] = x[b][kp]
        biask = np.where(np.arange(lk) < len(kp), 0.0, NEGB).astype(np.float32)
        xT = np.ascontiguousarray(x[b].T)           # [D, L]
        xt_l = (xT.reshape(8, 128, NQT, QT).transpose(2, 1, 0, 3)
                  .transpose(1, 0, 2, 3).reshape(128, -1))  # [128, NQT*8*QT]
        xkv_sw = _swiz(np.ascontiguousarray(xkv_b.T), 8)    # [128, 8, lk]
        xkv_fl = np.concatenate(
            [xkv_sw[:, :, 0:512].reshape(128, -1),
             xkv_sw[:, :, 512:1024].reshape(128, -1),
             xkv_sw[:, :, 1024:lk].reshape(128, -1)], axis=1)
        bin_ = np.concatenate([
            xt_l,
            xkv_fl,
            _swiz(np.ascontiguousarray(Wq[:, sl]), 8).reshape(128, -1),
            _swiz(np.ascontiguousarray(Wk[:, sl]), 8).reshape(128, -1),
            _swiz(np.ascontiguousarray(Wv[:, sl]), 8).reshape(128, -1),
            _swiz(np.ascontiguousarray(Wfc[sl, :]), 4).reshape(128, -1),
        ], axis=1).astype(bfloat16)
        fin = np.zeros((128, 16 + kck + HD), np.float32)
        fin[:, 0:4] = bq[sl].reshape(4, 128).T
        fin[:, 4:8] = bk[sl].reshape(4, 128).T
        fin[:, 8:16] = (bfc * 0.5).reshape(8, 128).T
        fin[:, 16:16 + kck] = biask.reshape(kck, 128).T
        fin[0, 16 + kck:] = bv[sl]
        in_maps.append({"bin": np.ascontiguousarray(bin_),
                        "fin": np.ascontiguousarray(fin)})
    return in_maps, lk


def _host_reference(x, mask, Wq, bq, Wk, bk, Wv, bv, Wfc, bfc):
    """Numpy fallback, bit-compatible with the reference semantics."""
    out = np.empty((BS, L, D), np.float32)
    for b in range(BS):
        q = (x[b] @ Wq + bq).reshape(L, 16, DK).transpose(1, 0, 2)
        k = (x[b] @ Wk + bk).reshape(L, 16, DK).transpose(1, 0, 2)
        v = (x[b] @ Wv + bv).reshape(L, 16, DK).transpose(1, 0, 2)
        s = np.einsum("hqd,hkd->hqk", q, k) * SCALE
        m = mask[b].astype(np.float32)[None, None, :]
        s = s * (1.0 - m) + m * (-1e30)
        s = s - s.max(axis=-1, keepdims=True)
        p = np.exp(s)
        p /= p.sum(axis=-1, keepdims=True)
        o = np.einsum("hqk,hkd->hqd", p, v).transpose(1, 0, 2).reshape(L, D)
        out[b] = o @ Wfc + bfc
    return out


def kernel(x, mask, Wq, bq, Wk, bk, Wv, bv, Wfc, bfc, **_unused):
    x = np.asarray(x, np.float32)
    mask = np.asarray(mask)
    Wq, bq = np.asarray(Wq, np.float32), np.asarray(bq, np.float32)
    Wk, bk = np.asarray(Wk, np.float32), np.asarray(bk, np.float32)
    Wv, bv = np.asarray(Wv, np.float32), np.asarray(bv, np.float32)
    Wfc, bfc = np.asarray(Wfc, np.float32), np.asarray(bfc, np.float32)

    in_maps, lk = _prep_in_maps(x, mask, Wq, bq, Wk, bk, Wv, bv, Wfc, bfc)
    if in_maps is None:
        return _host_reference(x, mask, Wq, bq, Wk, bk, Wv, bv, Wfc, bfc)
    results = _get_runner(lk).run(in_maps)

    out = np.empty((BS, L, D), np.float32)
    for b in range(BS):
        p0 = results[2 * b]["out"].reshape(D, L).astype(np.float32)
        p1 = results[2 * b + 1]["out"].reshape(D, L).astype(np.float32)
        out[b] = (p0 + p1).T
    return out
